# revision 56
# baseline (speedup 1.0000x reference)
"""RNN-T JointNet fused Bass kernel for Trainium2, SPMD over 8 NeuronCores.

Reference computation (all fp32):
    enc = LN(encoder_out @ W_enc + b_enc) * g_enc + be_enc      # [B,T,J]
    dec = LN(decoder_out @ W_dec + b_dec) * g_dec + be_dec      # [B,U,J]
    joint = relu(enc[:,:,None,:] + dec[:,None,:,:])             # [B,T,U,J]
    out = joint @ W_out + b_out                                 # [B,T,U,V]

Shapes: B=4, T=512, U=64, E=D=J=512, V=1024.

Sharding: data-parallel over the flattened (B,T) axis. Core c owns
b = c//2, t in [(c%2)*256, (c%2)*256+256) -> 16384 output rows, which are
contiguous in the flattened [B*T*U, V] output, so the gather is a concat.

All-bf16 on-chip pipeline (matmul accumulation stays fp32 in PSUM,
LayerNorm stats in fp32; rel err ~3e-3 vs the 2e-2 gate):
  - host pre-casts activations + weights to bf16 (halves input DMA bytes)
  - bf16 main GEMM streams at ~217 ns/MM (FWL weight loads fully hidden
    in the v-interleaved accumulation pattern; fp32r is 4x slower at peak
    clock and triggers throttling)
  - per supertile (512 rows x J): gpsimd broadcast-add -> DVE relu
    (tensor_scalar_max, 16-bit 2x mode) -> 32 bf16 matmuls -> PSUM
    evictions split DVE/ACT -> one 512 KiB DMA per 128-row block
  - input DMAs ordered by critical path: dec_x, enc_x, W_dec, W_enc, W_out
"""

import numpy as np

B, T, U = 4, 512, 64
E = D = J = 512
V = 1024
EPS = 1e-5
P = 128
NCORES = 8
TC = T * B // NCORES            # 256 t-rows per core
ROWS = TC * U                   # 16384 output rows per core
MM_TILES = ROWS // 512          # 32 supertiles of 512 rows (8 t values)
TSUP = 512 // U                 # 8 t values per supertile
KJ = J // P                     # 4 contraction blocks

_CACHE = {}


def _build(apply_b_enc, apply_g_enc, apply_be_enc,
           apply_b_dec, apply_g_dec, apply_be_dec, apply_b_out):
    import concourse.bass as bass
    import concourse.mybir as mybir
    import concourse.tile as tile
    from concourse import bacc
    from concourse.masks import make_identity

    f32 = mybir.dt.float32
    bf16 = mybir.dt.bfloat16
    AF = mybir.ActivationFunctionType
    OP = mybir.AluOpType

    nc = bacc.Bacc(target_bir_lowering=False)

    # activations/weights arrive pre-shuffled from the host into the exact
    # SBUF partition layout, so every DMA line is 1-8 KiB contiguous
    enc_x = nc.dram_tensor("enc_x", [P, TC // P, E], bf16, kind="ExternalInput")
    dec_x = nc.dram_tensor("dec_x", [U, D], bf16, kind="ExternalInput")
    w_enc = nc.dram_tensor("w_enc", [P, E // P, J], bf16, kind="ExternalInput")
    w_dec = nc.dram_tensor("w_dec", [P, D // P, J], bf16, kind="ExternalInput")
    w_out = nc.dram_tensor("w_out", [P, KJ, V], bf16, kind="ExternalInput")
    b_enc = nc.dram_tensor("b_enc", [J], f32, kind="ExternalInput")
    g_enc = nc.dram_tensor("g_enc", [J], f32, kind="ExternalInput")
    be_enc = nc.dram_tensor("be_enc", [J], f32, kind="ExternalInput")
    b_dec = nc.dram_tensor("b_dec", [J], f32, kind="ExternalInput")
    g_dec = nc.dram_tensor("g_dec", [J], f32, kind="ExternalInput")
    be_dec = nc.dram_tensor("be_dec", [J], f32, kind="ExternalInput")
    b_out = nc.dram_tensor("b_out", [V], f32, kind="ExternalInput")
    out = nc.dram_tensor("out", [ROWS, V], f32, kind="ExternalOutput")

    def bcast_row(dram_vec, n):
        # AP that reads a [n] DRAM vector replicated across 128 partitions
        return bass.AP(tensor=dram_vec.tensor, offset=dram_vec.offset,
                       ap=[[0, P], [1, n]])

    from contextlib import ExitStack

    with tile.TileContext(nc) as tc, ExitStack() as ctx:
        const = ctx.enter_context(tc.tile_pool(name="const", bufs=1))
        prep = ctx.enter_context(tc.tile_pool(name="prep", bufs=2))
        jpool = ctx.enter_context(tc.tile_pool(name="jpool", bufs=2))
        jrpool = ctx.enter_context(tc.tile_pool(name="jrpool", bufs=3))
        opool = ctx.enter_context(tc.tile_pool(name="opool", bufs=6))
        mpsum = ctx.enter_context(tc.tile_pool(name="mpsum", bufs=7, space="PSUM"))

        # input DMAs all on the sync queue so hardware-queue order == issue
        # order == critical-path order: dec_x, enc, W_dec, W_enc, W_out
        # (a second issuing engine would race its packets ahead of the
        # activations in the shared hardware queue)
        dx_sb = prep.tile([P, D], bf16, tag="dx_sb")
        nc.sync.dma_start(dx_sb[:U], dec_x[:])
        x_sb = prep.tile([P, TC // P, E], bf16, tag="x_sb")
        nc.sync.dma_start(x_sb[:], enc_x[:])

        wdec_sb = const.tile([P, D // P, J], bf16)
        wenc_sb = const.tile([P, E // P, J], bf16)
        wout_sb = const.tile([P, KJ, V], bf16)
        nc.sync.dma_start(wenc_sb[:], w_enc[:])
        nc.sync.dma_start(wdec_sb[:], w_dec[:])
        nc.sync.dma_start(wout_sb[:, 0:2], w_out[:, 0:2])
        nc.sync.dma_start(wout_sb[:, 2:4], w_out[:, 2:4])

        ident = const.tile([P, P], bf16)
        make_identity(nc, ident)
        eps_sb = const.tile([P, 1], f32)
        nc.vector.memset(eps_sb[:], EPS)

        # warm-up / gap-filler matmuls: the PE p-state ramp needs ~3us of
        # continuous execution to reach full clock, and any idle gap resets
        # it (half-rate matmuls for the next ~3us). Dummy matmuls burn the
        # unavoidable DMA-wait windows and keep the ramp alive.
        warm_src = const.tile([P, 512], bf16)
        nc.vector.memset(warm_src[:], 0.00390625)
        # dedicated PSUM bank: sharing the rotating pool would chain the
        # fillers onto unrelated readers and stall the PE mid-prep
        wpsum = ctx.enter_context(tc.tile_pool(name="wpsum", bufs=1, space="PSUM"))
        warm_ps = wpsum.tile([P, 512], f32)

        def warm(n):
            for _ in range(n):
                nc.tensor.matmul(warm_ps, warm_src[:, 0:P], warm_src[:],
                                 start=True, stop=True)

        # DVE and ACT have their own clock ramps (first big ops run ~5x
        # slow) that decay during idle; front-load dummy work and sprinkle
        # fillers through prep so they are at speed for the LN + first relus
        warm_dve = const.tile([P, 512], bf16)
        warm_act = const.tile([P, 512], bf16)

        def warm_v(n):
            for _ in range(n):
                nc.vector.tensor_scalar_max(warm_dve[:], warm_src[:], 0.0)

        def warm_a(n):
            for _ in range(n):
                nc.scalar.copy(warm_act[:], warm_src[:])

        warm_v(4)
        warm_a(3)


        def load_vec(vec, n, enabled):
            if not enabled:
                return None
            t = const.tile([P, n], f32)
            nc.sync.dma_start(t[:], bcast_row(vec, n))
            return t

        b_enc_sb = load_vec(b_enc, J, apply_b_enc)
        g_enc_sb = load_vec(g_enc, J, apply_g_enc)
        be_enc_sb = load_vec(be_enc, J, apply_be_enc)
        b_dec_sb = load_vec(b_dec, J, apply_b_dec)
        g_dec_sb = load_vec(g_dec, J, apply_g_dec)
        be_dec_sb = load_vec(be_dec, J, apply_be_dec)
        b_out_sb = load_vec(b_out, V, apply_b_out)

        encT = const.tile([P, KJ, TC], bf16)
        decT = const.tile([P, KJ, U], bf16)

        def layer_norm_rows(x_f32, out_b16, rows, g_sb, be_sb, who,
                            norm_on_act=True):
            # x_f32: [rows, J] fp32 in SBUF -> normalized bf16 in out_b16
            stats = prep.tile([P, 6], f32, tag="ln_stats", name=f"st_{who}")
            mv = prep.tile([P, 2], f32, tag="ln_mv", name=f"mv_{who}")
            nc.vector.bn_stats(out=stats[:rows], in_=x_f32[:rows])
            nc.vector.bn_aggr(out=mv[:rows], in_=stats[:rows])
            rstd = prep.tile([P, 1], f32, tag="ln_rstd", name=f"rs_{who}")
            nc.scalar.activation(out=rstd[:rows], in_=mv[:rows, 1:2],
                                 func=AF.Sqrt, bias=eps_sb[:rows], scale=1.0)
            nc.vector.reciprocal(out=rstd[:rows], in_=rstd[:rows])
            if g_sb is None and be_sb is None:
                if norm_on_act:
                    # normalize on ACT: out = x*rstd + (-mu*rstd); reads
                    # PSUM directly; the dec norm stays on DVE so both
                    # normalizes run in parallel
                    nb = prep.tile([P, 1], f32, tag="ln_nb", name=f"nb_{who}")
                    nc.vector.tensor_scalar(nb[:rows], mv[:rows, 0:1],
                                            rstd[:rows], -1.0, OP.mult, OP.mult)
                    nc.scalar.activation(out=out_b16[:rows], in_=x_f32[:rows],
                                         func=AF.Identity, bias=nb[:rows],
                                         scale=rstd[:rows])
                else:
                    nc.vector.tensor_scalar(out_b16[:rows], x_f32[:rows],
                                            mv[:rows, 0:1], rstd[:rows],
                                            OP.subtract, OP.mult)
            else:
                tmp = prep.tile([P, J], f32, tag="ln_tmp", name=f"tmp_{who}")
                nc.vector.tensor_scalar(tmp[:rows], x_f32[:rows],
                                        mv[:rows, 0:1], rstd[:rows],
                                        OP.subtract, OP.mult)
                if g_sb is not None:
                    nc.vector.tensor_mul(tmp[:rows], tmp[:rows], g_sb[:rows])
                if be_sb is not None:
                    nc.vector.tensor_add(tmp[:rows], tmp[:rows], be_sb[:rows])
                nc.scalar.copy(out_b16[:rows], tmp[:rows])

        # ---- prep, ordered to keep the PE streaming continuously (each gap
        # restarts the ~3us p-state ramp) and to start supertile 0 ASAP ----
        def in_transposes(tb):
            xT = prep.tile([P, E // P, P], bf16, tag="xT", name=f"xT_{tb}")
            for k in range(E // P):
                pt = mpsum.tile([P, P], bf16, tag="mps", name=f"ept_{tb}_{k}")
                nc.tensor.transpose(pt[:], x_sb[:, tb, k * P:(k + 1) * P], ident[:])
                nc.vector.tensor_copy(xT[:, k, :], pt[:])
            return xT

        def enc_proj(tb, xT):
            eps_mm = mpsum.tile([P, J], f32, tag="mps", name=f"emm_{tb}")
            for k in range(E // P):
                nc.tensor.matmul(eps_mm[:], xT[:, k, :], wenc_sb[:, k, :],
                                 start=(k == 0), stop=(k == E // P - 1))
            return eps_mm

        def enc_ln(tb, eps_mm, direct):
            encln = prep.tile([P, J], bf16, tag="encln", name=f"encln_{tb}")
            if b_enc_sb is None and direct:
                # LN straight off PSUM: skips the copy on the critical path
                layer_norm_rows(eps_mm, encln, P, g_enc_sb, be_enc_sb, f"enc{tb}")
                return encln
            encf = prep.tile([P, J], f32, tag="encf", name=f"encf_{tb}")
            if b_enc_sb is not None:
                nc.vector.tensor_add(encf[:], eps_mm[:], b_enc_sb[:])
            else:
                nc.vector.tensor_copy(encf[:], eps_mm[:])
            layer_norm_rows(encf, encln, P, g_enc_sb, be_enc_sb, f"enc{tb}")
            return encln

        def enc_out_transposes(tb, encln):
            for jb in range(KJ):
                pt = mpsum.tile([P, P], bf16, tag="mps", name=f"eot_{tb}_{jb}")
                nc.tensor.transpose(pt[:], encln[:, jb * P:(jb + 1) * P],
                                    ident[:])
                nc.vector.tensor_copy(encT[:, jb, tb * P:(tb + 1) * P], pt[:])

        # ramp the PE clock while the first input DMAs land
        warm(12)
        # decoder transposes (first PE work, only needs dec_x)
        dxT = prep.tile([P, D // P, U], bf16, tag="dxT")
        for k in range(D // P):
            pt = mpsum.tile([P, P], bf16, tag="mps", name=f"dpt_{k}")
            nc.tensor.transpose(pt[:, :U], dx_sb[:U, k * P:(k + 1) * P],
                                ident[:U, :U])
            nc.vector.tensor_copy(dxT[:, k, :], pt[:, :U])
        warm_v(1)
        xT0 = in_transposes(0)
        warm_v(1)
        # enc block 0 first (its proj+LN+transposes are supertile 0's long
        # pole; W_enc is ordered ahead of W_dec in the DMA queue to match)
        emm0 = enc_proj(0, xT0)
        dps = mpsum.tile([P, J], f32, tag="mps", name="dps")
        for k in range(D // P):
            nc.tensor.matmul(dps[:U], dxT[:, k, :], wdec_sb[:, k, :],
                             start=(k == 0), stop=(k == D // P - 1))
        encln0 = enc_ln(0, emm0, direct=True)
        # dec normalize on DVE while enc0's runs on ACT -> chains overlap
        decln = prep.tile([P, J], bf16, tag="decln")
        if b_dec_sb is None:
            layer_norm_rows(dps, decln, U, g_dec_sb, be_dec_sb, "dec",
                            norm_on_act=False)
        else:
            decf = prep.tile([P, J], f32, tag="decf")
            nc.vector.tensor_add(decf[:U], dps[:U], b_dec_sb[:U])
            layer_norm_rows(decf, decln, U, g_dec_sb, be_dec_sb, "dec")
        xT1 = in_transposes(1)
        warm_a(1)
        warm(5)   # cover the enc block-0 LN latency
        enc_out_transposes(0, encln0)
        for jb in range(KJ):
            pt = mpsum.tile([P, P], bf16, tag="mps", name=f"dot_{jb}")
            nc.tensor.transpose(pt[:, :U], decln[:U, jb * P:(jb + 1) * P],
                                ident[:U, :U])
            nc.vector.tensor_copy(decT[:, jb, :], pt[:, :U])
        # block-1 projection keeps the PE warm while supertile 0's joint is
        # built; its LN + out-transposes are emitted after supertile 0 below
        emm1 = enc_proj(1, xT1)
        warm(9)   # cover supertile 0's first jr-chunk latency

        # ---- main loop: 32 supertiles x 512 rows ----
        # evictions alternate DVE/ACT (4 each); relu halves also split
        # across DVE/ACT, so both stay well under the 7us supertile budget
        # ACT takes all early evictions; DVE's (behind its two relus in
        # FIFO) sit late in the supertile where bank-reuse slack is largest
        evict_on_act = (True, True, True, False, True, False, True, False)
        out_r = out[:].rearrange("(mm j p) v -> mm p j v", j=KJ, p=P)
        for mm in range(MM_TILES):
            joint = jpool.tile([P, KJ, 512], bf16, tag="joint")
            jr = jrpool.tile([P, KJ, 512], bf16, tag="jr")
            jv = joint.rearrange("p k (t u) -> p k t u", u=U)
            # joint built in chunks (broadcast-add -> relu): finer chunks
            # for the first supertiles (latency-critical, adds split across
            # gpsimd+DVE to halve serial latency), halves otherwise
            nch = 4 if mm < 2 else 2
            tw = TSUP // nch
            for ch in range(nch):
                tsl = slice(mm * TSUP + ch * tw, mm * TSUP + (ch + 1) * tw)
                csl = slice(ch * tw * U, (ch + 1) * tw * U)
                enc_b = encT[:, :, tsl, None].to_broadcast((P, KJ, tw, U))
                dec_b = decT[:, :, None, :].to_broadcast((P, KJ, tw, U))
                add_eng = nc.vector if (mm < 2 and ch % 2 == 1) else nc.gpsimd
                add_eng.tensor_tensor(jv[:, :, ch * tw:(ch + 1) * tw],
                                      dec_b, enc_b, OP.add)
                # all relus on DVE (415ns/half there vs 1148 on ACT); an ACT
                # relu pushes an eviction late and the PE then waits ~215ns
                # on the recycled PSUM bank every other supertile
                nc.vector.tensor_scalar_max(jr[:, :, csl],
                                            joint[:, :, csl], 0.0)
            if mm == 0:
                # block-1 LN must be emitted before supertile 0's PSUM
                # allocations recycle emm1's bank (its DVE reads precede the
                # evictions in queue order; the work hides under the matmuls)
                encln1 = enc_ln(1, emm1, direct=False)
            for j in range(KJ):
                stage = opool.tile([P, V], f32, tag="stage", name=f"st_{mm}_{j}")
                pss = [mpsum.tile([P, 512], f32, tag="mps",
                                  name=f"ps_{mm}_{j}_{v}") for v in range(2)]
                for k in range(KJ):
                    for v in range(2):
                        nc.tensor.matmul(
                            pss[v],
                            jr[:, k, j * P:(j + 1) * P],
                            wout_sb[:, k, v * 512:(v + 1) * 512],
                            start=(k == 0), stop=(k == KJ - 1))
                for v in range(2):
                    dst = stage[:, v * 512:(v + 1) * 512]
                    if b_out_sb is not None:
                        nc.vector.tensor_add(dst, pss[v][:],
                                             b_out_sb[:, v * 512:(v + 1) * 512])
                    elif evict_on_act[j * 2 + v]:
                        nc.scalar.copy(dst, pss[v][:])
                    else:
                        nc.vector.tensor_copy(dst, pss[v][:])
                nc.sync.dma_start(out_r[mm, :, j], stage[:])
            if mm == 0:
                # block-1 out-transposes slot in behind supertile 0's matmuls
                enc_out_transposes(1, encln1)

    nc.compile()
    return nc


def kernel(**inputs):
    import ml_dtypes
    from concourse.bass_utils import run_bass_kernel_spmd

    bf = ml_dtypes.bfloat16
    enc = np.ascontiguousarray(np.asarray(inputs["encoder_out"], dtype=np.float32).astype(bf))
    dec = np.ascontiguousarray(np.asarray(inputs["decoder_out"], dtype=np.float32).astype(bf))

    def shuf(w):
        # [(o p), n] row-blocked -> [p, o, n] partition-major SBUF layout
        o = w.shape[0] // P
        return np.ascontiguousarray(w.reshape(o, P, w.shape[1]).transpose(1, 0, 2))

    named = {}
    for k_src, k_dst in [("W_enc", "w_enc"), ("W_dec", "w_dec"), ("W_out", "w_out")]:
        named[k_dst] = shuf(np.asarray(inputs[k_src], dtype=np.float32).astype(bf))
    for k_src, k_dst in [("b_enc", "b_enc"), ("g_enc", "g_enc"), ("be_enc", "be_enc"),
                         ("b_dec", "b_dec"), ("g_dec", "g_dec"), ("be_dec", "be_dec"),
                         ("b_out", "b_out")]:
        named[k_dst] = np.ascontiguousarray(np.asarray(inputs[k_src], dtype=np.float32))

    flags = (
        bool(np.any(named["b_enc"])), not np.all(named["g_enc"] == 1.0),
        bool(np.any(named["be_enc"])),
        bool(np.any(named["b_dec"])), not np.all(named["g_dec"] == 1.0),
        bool(np.any(named["be_dec"])),
        bool(np.any(named["b_out"])),
    )
    if flags not in _CACHE:
        _CACHE[flags] = _build(*flags)
    nc = _CACHE[flags]

    tpc = T // (NCORES // B)      # t-rows per core
    in_maps = []
    for c in range(NCORES):
        b = c // (NCORES // B)
        t0 = (c % (NCORES // B)) * tpc
        in_maps.append({
            "enc_x": shuf(enc[b, t0:t0 + tpc]),
            "dec_x": np.ascontiguousarray(dec[b]),
            **named,
        })

    res = run_bass_kernel_spmd(nc, in_maps, core_ids=list(range(NCORES)))
    full = np.concatenate([res.results[c]["out"] for c in range(NCORES)], axis=0)
    return full.reshape(B, T, U, V)
